# revision 25
# baseline (speedup 1.0000x reference)
# Causal self-attention (B=2, S=2048, D=1024, H=16) on 8 TRN2 NeuronCores.
#
# Sharding: core = (batch b, head-group hg) with 4 heads per core — data
# parallel on B (cores 0-3 = batch 0, cores 4-7 = batch 1), tensor parallel
# on heads within each batch group. Per core:
#   1. qkv^T projection for its 4 heads:  qkvT[768, 2048] = wqkv_s^T @ x_b^T
#   2. causal attention in scores^T layout (keys on partitions), software-
#      pipelined one k-tile deep so the PE never waits on the ACT exp:
#        scoresT[k, q] = K^T.T @ Q^T ; exp on ACT (scale=1/8 fused);
#        diagonal-block masking via an upper-triangular mask multiply;
#        ctxT[d, q] accumulates with a ones-column appended to V so row 64
#        of the ctx psum is the softmax denominator.
#   3. normalize: ONE [4,512] reciprocal for all 4 heads' denominators,
#      broadcast via two indicator-matrix matmuls, two DVE multiplies
#   4. AllGather ctx^T over the 4-core batch group
#   5. out-projection of chunk n-1 is emitted after gather(n) so the PE's
#      in-order queue never head-of-line blocks on a collective
# Host side shards/pre-transposes inputs and concatenates the 8 output
# column-slices; no host arithmetic beyond dtype casts and transposes.

import numpy as np
import ml_dtypes

import concourse.bass as bass
import concourse.mybir as mybir
import concourse.tile as tile
from concourse import bacc
from concourse.bass_utils import run_bass_kernel_spmd
from concourse.masks import make_identity, make_upper_triangular

F32 = mybir.dt.float32
BF16 = mybir.dt.bfloat16

B, S, D, H, HD = 2, 2048, 1024, 16, 64
HG = 4                 # heads per core
DG = HG * HD           # 256 qkv cols per head-group
NCORES = 8
KT = 128               # key tile (partition dim of scoresT)
QC = 512               # query chunk (free dim of scoresT / psum width)
NKT = S // KT          # 16 key tiles
NQC = S // QC          # 4 query chunks
SM_SCALE = 1.0 / 8.0   # 1/sqrt(HD)

# dtype knobs (matmul operand / storage dtypes; psums always fp32)
XW_DT = BF16           # x and w_qkv (qkv projection matmul)
QK8 = mybir.dt.float8e4  # Q^T/K^T tiles (DoubleRow scores matmul)
V_DT = BF16            # V natural tiles (ctx matmul lhsT)
ATTN_DT = BF16         # exp(scores) tiles (ctx matmul rhs)
CC_DT = BF16           # allgathered ctx^T
WOUT_DT = BF16         # out-projection weights
OUT_DT = BF16          # final output (host casts back to f32)

_NP = {BF16: ml_dtypes.bfloat16, F32: np.float32}

LAST_RESULTS = None    # BassKernelResults of the most recent kernel() call
_NC_CACHE = {}


def _build_nc():
    nc = bacc.Bacc(
        trn_type="TRN2",
        target_bir_lowering=False,
        debug=False,
        num_devices=NCORES,
    )

    xT = nc.declare_dram_parameter("xT", [D, S], XW_DT, isOutput=False)
    wqkv = nc.declare_dram_parameter("wqkv", [D, 3 * DG], XW_DT, isOutput=False)
    bqkv = nc.declare_dram_parameter("bqkv", [128, 6], F32, isOutput=False)
    wout = nc.declare_dram_parameter("wout", [D, DG], WOUT_DT, isOutput=False)
    bout = nc.declare_dram_parameter("bout", [128, 2], F32, isOutput=False)
    outT = nc.declare_dram_parameter("outT", [DG, S], OUT_DT, isOutput=True)

    KC = D // 128  # 8 contraction chunks for the projections

    with tile.TileContext(nc) as tc:
        with tc.tile_pool(name="persist", bufs=1) as ps:
            # ---- constants ----
            identity = ps.tile([128, 128], XW_DT, tag="identity")
            make_identity(nc, identity)
            tri = ps.tile([128, 128], F32, tag="tri")
            make_upper_triangular(nc, tri, val=1.0, diag=True)
            tri_mm = ps.tile([128, 128], ATTN_DT, tag="tri_mm")
            nc.vector.tensor_copy(tri_mm, tri)
            # indicator matrices for the denominator broadcast: head h's
            # sums/recip live on partition 32h (engines can only address
            # partitions starting at 0/32/64/96).
            # bc_mv[p, q] = recip4[32 * (2*mv + (p >= 64)), q]
            ind = ps.tile([128, 256], ATTN_DT, tag="ind")
            nc.vector.memset(ind, 0.0)
            nc.vector.memset(ind[0:1, 0:64], 1.0)
            nc.vector.memset(ind[32:33, 64:128], 1.0)
            nc.vector.memset(ind[64:65, 128 + 0:128 + 64], 1.0)
            nc.vector.memset(ind[96:97, 128 + 64:128 + 128], 1.0)
            sums4 = ps.tile([128, QC], F32, tag="sums4")
            nc.vector.memset(sums4, 1.0)
            recip4 = ps.tile([128, QC], ATTN_DT, tag="recip4")
            raw2 = ps.tile([128, 2, QC], F32, tag="raw2")

            # ---- persistent SBUF tensors ----
            xT_sb = ps.tile([128, KC, S], XW_DT, tag="xT_sb")
            wqkv_sb = ps.tile([128, KC, 3 * DG], XW_DT, tag="wqkv_sb")
            bqkv_sb = ps.tile([128, 6], F32, tag="bqkv_sb")
            # Q^T/K^T in fp8 DoubleRow layout: head h's dims live at
            # partitions 32h..32h+31, split into two 32-dim subtiles along
            # a free axis (dim d of head h -> [32h + d % 32, qk, d // 32, s])
            qk8_sb = ps.tile([128, 2, 2, S], QK8, tag="qk8_sb")
            vT_sb = ps.tile([128, 2, S], V_DT, tag="vT_sb")       # V^T
            vnat_sb = ps.tile([128, HG, NKT, HD + 1], V_DT, tag="vnat_sb")
            ctx_sb = ps.tile([128, 2, S], CC_DT, tag="ctx_sb")    # normalized
            ctxg_sb = ps.tile([128, D // 128, S], CC_DT, tag="ctxg_sb")
            wout_sb = ps.tile([128, KC, DG], WOUT_DT, tag="wout_sb")
            bout_sb = ps.tile([128, 2], F32, tag="bout_sb")
            outT_sb = ps.tile([128, 2, S], OUT_DT, tag="outT_sb")

            # ---- load inputs ----
            # weights + chunk-0 columns of x first so proj(0) starts early
            xT_r = xT.rearrange("(c p) s -> c p s", p=128)
            wqkv_r = wqkv.rearrange("(c p) m -> c p m", p=128)
            wout_r = wout.rearrange("(c p) m -> c p m", p=128)
            nc.sync.dma_start(out=bqkv_sb, in_=bqkv[:])
            for c in range(KC):
                nc.sync.dma_start(out=wqkv_sb[:, c, :], in_=wqkv_r[c])
            for c in range(KC):
                nc.sync.dma_start(out=xT_sb[:, c, 0:QC], in_=xT_r[c, :, 0:QC])
            for c in range(KC):
                nc.sync.dma_start(out=xT_sb[:, c, QC:S], in_=xT_r[c, :, QC:S])
                nc.sync.dma_start(out=wout_sb[:, c, :], in_=wout_r[c])
            nc.sync.dma_start(out=bout_sb, in_=bout[:])

            # ---- stages 1-4 interleaved per token chunk:
            # proj(n) -> V-transpose(n) -> attention(n) -> gather(n) ->
            # out_proj(n-1).  out-proj is delayed one chunk so its PE
            # instructions sit behind attention(n) in the in-order queue,
            # giving gather(n-1) a whole chunk of compute to land under.
            nc.vector.memset(vnat_sb[:, :, :, HD:HD + 1], 1.0)
            with tc.tile_pool(name="dram", bufs=1, space="DRAM") as dram:

                cc_in = [[dram.tile([128, QC], CC_DT, tag=f"cc_in{q}_{b}",
                                    name=f"cc_in{q}_{b}") for b in range(2)]
                         for q in range(NQC)]
                cc_out = [[dram.tile([512, QC], CC_DT, tag=f"cc_out{q}_{b}",
                                     name=f"cc_out{q}_{b}") for b in range(2)]
                          for q in range(NQC)]

                def proj_filler(n):
                    """Yield one closure per PE instruction of proj(n).
                    m-chunk order q01 k01 q23 k23 v01 v23 so head 0/1's
                    attention deps complete first; V transposes last."""
                    for m in (0, 2, 1, 3, 4, 5):
                        pt = [None]

                        def mm(c, m=m, pt=pt):
                            if c == 0:
                                pt[0] = gpp.tile([128, QC], F32, tag="gemm",
                                                 name="pj_ps")
                            nc.tensor.matmul(
                                pt[0],
                                lhsT=wqkv_sb[:, c, m * 128:(m + 1) * 128],
                                rhs=xT_sb[:, c, n * QC:(n + 1) * QC],
                                start=(c == 0),
                                stop=(c == KC - 1),
                            )
                            if c == KC - 1:
                                cols = slice(n * QC, (n + 1) * QC)
                                if m < 4:
                                    # bias + fp8 quantize + regroup: psum row
                                    # 64*hh + 32*sub (head pair hh, subtile
                                    # sub) -> qk8 partition 32*h, free (qk,
                                    # sub).  m 0,1 = q heads (01),(23);
                                    # m 2,3 = k likewise.
                                    qk = m // 2
                                    hb = 64 * (m % 2)   # head base: h0/h2
                                    with nc.allow_low_precision(
                                            reason="fp8 scores operands"):
                                        for hh in range(2):
                                            for sub in range(2):
                                                pr = 64 * hh + 32 * sub
                                                tb = 32 * (hb // 32 + hh)
                                                nc.vector.tensor_scalar_add(
                                                    qk8_sb[tb:tb + 32, qk,
                                                           sub, cols],
                                                    pt[0][pr:pr + 32, :],
                                                    bqkv_sb[pr:pr + 32,
                                                            m:m + 1])
                                else:
                                    nc.vector.tensor_scalar_add(
                                        vT_sb[:, m - 4, cols], pt[0],
                                        bqkv_sb[:, m:m + 1])
                        for c in range(KC):
                            yield lambda c=c, mm=mm: mm(c)
                    # V natural (+ ones column) via PE transpose
                    for h in range(HG):
                        po = 64 * (h % 2)
                        mv = h // 2
                        for t in range(4 * n, 4 * n + 4):
                            def vt(h=h, po=po, mv=mv, t=t):
                                tp = axp.tile(
                                    [128, HD], V_DT, tag="tp", bufs=1,
                                    name="tp")
                                nc.tensor.transpose(
                                    tp,
                                    vT_sb[po:po + 64, mv, t * KT:(t + 1) * KT],
                                    identity[po:po + 64, po:po + 64],
                                )
                                nc.vector.tensor_copy(
                                    vnat_sb[:, h, t, 0:HD], tp)
                            yield vt

                def attention_chunk(j, fillers, late=None, late_start=0):
                    """Emit attention for chunk j, interleaving filler PE
                    instructions (projection / out-proj work for other
                    chunks) into the ACT-bound inner loop.  `late` fillers
                    are only absorbed from tile `late_start` on (their data
                    dependency — a collective — needs time to land)."""
                    n_kt = 4 * j + 4      # key tiles 0 .. 4j+3
                    t_idx = [0]

                    def absorb(k):
                        for _ in range(k):
                            f = next(fillers, None)
                            if f is None and late is not None \
                                    and t_idx[0] >= late_start:
                                f = next(late, None)
                            if f is None:
                                return
                            f()

                    for h in range(HG):
                        po = 64 * (h % 2)
                        mh = h // 2
                        cx = cxp.tile([HD + 1, QC], F32, tag="ctx")

                        def sc_exp(i):
                            # scores matmul + exp + diagonal mask for tile i
                            tshift = KT * i - QC * j
                            t0 = max(tshift, 0)
                            sc = scp.tile([128, QC], F32, tag="sc")
                            at = asb.tile([128, QC], ATTN_DT, tag="attn")
                            nc.tensor.matmul(
                                sc[:, t0:QC],
                                lhsT=qk8_sb[32 * h:32 * h + 32, 1, :,
                                            i * KT:(i + 1) * KT],
                                rhs=qk8_sb[32 * h:32 * h + 32, 0, :,
                                           j * QC + t0:(j + 1) * QC],
                                start=True, stop=True,
                                perf_mode=mybir.MatmulPerfMode.DoubleRow,
                                tile_position=(32 * h, 0),
                            )
                            nc.scalar.activation(
                                at[:, t0:QC], sc[:, t0:QC],
                                mybir.ActivationFunctionType.Exp,
                                scale=SM_SCALE,
                            )
                            if tshift >= 0:   # diagonal: mask k > q
                                nc.vector.tensor_mul(
                                    at[:, t0:t0 + 128],
                                    at[:, t0:t0 + 128], tri_mm)
                            return t0, at

                        def ctx_mm(i, t0, at):
                            nc.tensor.matmul(
                                cx[:, t0:QC],
                                lhsT=vnat_sb[:, h, i, :],
                                rhs=at[:, t0:QC],
                                start=(i == 0),
                                stop=(i == n_kt - 1),
                            )

                        # 1-tile software pipeline: the PE runs sc(i+1)
                        # while ACT computes exp(i), then ctx(i); one filler
                        # per tile rides the ACT slack.
                        pend = sc_exp(0)
                        for i in range(1, n_kt):
                            nxt = sc_exp(i)
                            ctx_mm(i - 1, *pend)
                            t_idx[0] += 1
                            absorb(2)
                            pend = nxt
                        ctx_mm(n_kt - 1, *pend)

                        # evict raw ctx + sums fast (frees the psum so the
                        # next head's k-loop isn't gated on normalization)
                        nc.vector.tensor_copy(
                            sums4[32 * h:32 * h + 1, :], cx[HD:HD + 1, :])
                        nc.vector.tensor_copy(
                            raw2[po:po + 64, mh, :], cx[0:HD, :])
                        if h % 2 == 1:
                            # both heads of m-block h//2 are done: normalize
                            # (per-block reciprocal + indicator-matmul
                            # broadcast + multiply) and launch this block's
                            # AllGather so it flies under remaining compute
                            mv = h // 2
                            with nc.allow_low_precision(
                                    reason="softmax denominator broadcast"):
                                nc.vector.reciprocal(
                                    recip4[64 * mv:64 * mv + 64, :],
                                    sums4[64 * mv:64 * mv + 64, :])
                            bc = bcp.tile([128, QC], F32, tag="bc")
                            nc.tensor.matmul(
                                bc,
                                lhsT=ind[64 * mv:64 * mv + 64,
                                         mv * 128:(mv + 1) * 128],
                                rhs=recip4[64 * mv:64 * mv + 64, :],
                                start=True, stop=True)
                            nc.vector.tensor_mul(
                                ctx_sb[:, mv, j * QC:(j + 1) * QC],
                                raw2[:, mv, :], bc)
                            gather_block(j, mv)

                def gather_block(q, blk):
                    # AllGather one m-block (2 heads) of chunk q's ctx: the
                    # [128, QC] block from each of the 4 cores lands in the
                    # interleaved ctxg chunks 2g+blk.
                    lo = q * QC
                    nc.sync.dma_start(
                        out=cc_in[q][blk][:], in_=ctx_sb[:, blk, lo:lo + QC])
                    nc.gpsimd.collective_compute(
                        "AllGather",
                        mybir.AluOpType.bypass,
                        replica_groups=[[0, 1, 2, 3], [4, 5, 6, 7]],
                        ins=[cc_in[q][blk][:].opt()],
                        outs=[cc_out[q][blk][:].opt()],
                    )
                    cc_out_r = cc_out[q][blk].rearrange(
                        "(g p) s -> g p s", p=128)
                    for g in range(4):
                        nc.sync.dma_start(
                            out=ctxg_sb[:, 2 * g + blk, lo:lo + QC],
                            in_=cc_out_r[g])

                def _op_mm(n, pts, mo, k):
                    # ctxg c-chunks 0,2,4,.. land with gather-a and 1,3,5,..
                    # with gather-b, so the accumulation runs evens first
                    c = (0, 2, 4, 6, 1, 3, 5, 7)[k]
                    if k == 0:
                        pts[mo] = gpp.tile([128, QC], F32, tag="gemm",
                                           name="op_ps")
                    nc.tensor.matmul(
                        pts[mo],
                        lhsT=wout_sb[:, c, mo * 128:(mo + 1) * 128],
                        rhs=ctxg_sb[:, c, n * QC:(n + 1) * QC],
                        start=(k == 0),
                        stop=(k == KC - 1),
                    )
                    if k == KC - 1:
                        outT_r = outT.rearrange("(c p) s -> c p s", p=128)
                        nc.vector.tensor_scalar_add(
                            outT_sb[:, mo, n * QC:(n + 1) * QC],
                            pts[mo], bout_sb[:, mo:mo + 1])
                        nc.sync.dma_start(
                            out=outT_r[mo, :, n * QC:(n + 1) * QC],
                            in_=outT_sb[:, mo, n * QC:(n + 1) * QC])

                def op_filler(n):
                    """One closure per PE instruction of out_proj(n), bias +
                    output DMA attached to the accumulation-group tails."""
                    pts = [None, None]
                    for mo in range(2):
                        for k in range(KC):
                            yield lambda n=n, pts=pts, mo=mo, k=k: \
                                _op_mm(n, pts, mo, k)

                def op_split(n):
                    """out_proj(n) split into a gather-a-only phase and the
                    rest, sharing accumulation psums across the phases."""
                    pts = [None, None]
                    a = [(mo, k) for mo in range(2) for k in range(4)]
                    b = [(mo, k) for mo in range(2) for k in range(4, KC)]
                    mk = (lambda mo, k:
                          (lambda: _op_mm(n, pts, mo, k)))
                    return ([mk(mo, k) for mo, k in a],
                            [mk(mo, k) for mo, k in b])

                with tc.tile_pool(name="gemm_ps", bufs=2, space="PSUM") as gpp, \
                     tc.tile_pool(name="aux_ps", bufs=1, space="PSUM") as axp, \
                     tc.tile_pool(name="bc_ps", bufs=1, space="PSUM") as bcp, \
                     tc.tile_pool(name="sc_ps", bufs=2, space="PSUM") as scp, \
                     tc.tile_pool(name="ctx_ps", bufs=2, space="PSUM") as cxp, \
                     tc.tile_pool(name="attn_sb", bufs=6) as asb:
                    # proj work for chunks 1..3 is a single filler stream,
                    # drained greedily into the attention inner loops; the
                    # per-chunk sentinel forces any un-absorbed remainder of
                    # proj(n+1) to be emitted before attention(n+1) starts.
                    def filler_stream():
                        for n in range(1, NQC):
                            yield ("chunk", n)
                            yield from proj_filler(n)

                    fs = filler_stream()
                    pending = []   # at most one lookahead item

                    def fillers_for(limit):
                        while True:
                            it = pending.pop() if pending else next(fs, None)
                            if it is None:
                                return
                            if isinstance(it, tuple):
                                if it[1] > limit:
                                    pending.append(it)
                                    return
                                continue
                            yield it

                    from itertools import chain as _chain
                    for g in proj_filler(0):
                        g()
                    for n in range(NQC):
                        # out-proj work rides the attention inner loops two
                        # chunks after its gather launched; chunk 3's own
                        # out-proj is split so its gather-a half runs while
                        # gather-b is still in flight
                        if n == 2:
                            late = op_filler(0)
                        elif n == 3:
                            late = _chain(op_filler(1), op_filler(2))
                        else:
                            late = None
                        attention_chunk(n, fillers_for(n + 1), late=late,
                                        late_start=8 if n == 3 else 0)
                        if late is not None:
                            for g in late:
                                g()
                        # any un-absorbed proj(n+1) must be emitted now
                        for g in fillers_for(n + 1):
                            g()
                    op3a, op3b = op_split(NQC - 1)
                    for g in op3a:
                        g()
                    for g in op3b:
                        g()

    nc.compile()
    return nc


def get_nc():
    if "nc" not in _NC_CACHE:
        _NC_CACHE["nc"] = _build_nc()
    return _NC_CACHE["nc"]


def make_in_maps(x, w_qkv, b_qkv, w_out, b_out):
    x = np.asarray(x, np.float32)
    w_qkv = np.asarray(w_qkv, np.float32)
    b_qkv = np.asarray(b_qkv, np.float32)
    w_out = np.asarray(w_out, np.float32)
    b_out = np.asarray(b_out, np.float32)

    xw_np = _NP[XW_DT]
    wout_np = _NP[WOUT_DT]

    xT = [np.ascontiguousarray(x[b].T).astype(xw_np) for b in range(B)]
    in_maps = []
    for core in range(NCORES):
        b, hg = core // HG, core % HG
        sl = slice(hg * DG, (hg + 1) * DG)
        wq = w_qkv[:, sl]
        wk = w_qkv[:, D + hg * DG:D + (hg + 1) * DG]
        wv = w_qkv[:, 2 * D + hg * DG:2 * D + (hg + 1) * DG]
        wqkv_s = np.ascontiguousarray(
            np.concatenate([wq, wk, wv], axis=1)).astype(xw_np)
        bq = np.concatenate(
            [b_qkv[sl], b_qkv[D + hg * DG:D + (hg + 1) * DG],
             b_qkv[2 * D + hg * DG:2 * D + (hg + 1) * DG]])
        in_maps.append({
            "xT": xT[b],
            "wqkv": wqkv_s,
            "bqkv": np.ascontiguousarray(bq.reshape(6, 128).T).astype(np.float32),
            "wout": np.ascontiguousarray(w_out[:, sl]).astype(wout_np),
            "bout": np.ascontiguousarray(
                b_out[sl].reshape(2, 128).T).astype(np.float32),
        })
    return in_maps


def assemble_output(results):
    out = np.empty((B, S, D), np.float32)
    for core in range(NCORES):
        b, hg = core // HG, core % HG
        out[b, :, hg * DG:(hg + 1) * DG] = \
            np.asarray(results[core]["outT"], dtype=np.float32).T
    return out


def kernel(x, w_qkv, b_qkv, w_out, b_out):
    global LAST_RESULTS
    in_maps = make_in_maps(x, w_qkv, b_qkv, w_out, b_out)
    nc = get_nc()
    res = run_bass_kernel_spmd(nc, in_maps, list(range(NCORES)))
    LAST_RESULTS = res
    return assemble_output(res.results)
